# revision 10
# baseline (speedup 1.0000x reference)
"""Trainium2 Bass kernel for nn_ContextAttention_21457656611319.

Reference math (per batch n):
    xf = x[n] reshaped [C, L], L = H*W = 4096
    q = Wq@xf + bq ; k = Wk@xf + bk ; v = Wv@xf + bv          [C, L]
    S[l,m] = sum_c k[c,l] q[c,m] * (1/sqrt(C))                 [L, L]
    T = softmax(S, axis=m)  (softmax over the m axis)
    attn[c,m] = sum_l v[c,l] T[l,m]
    out = x + attn

Sharding: 8 cores = 4 batches x 2-way shard of the l (key/value) axis.
Softmax rows (fixed l, all m) stay intact on one core, so each core
computes a partial attn (partial sum over its l-half) independently;
the host adds the two halves per batch plus x.  No collectives needed.

Per-core schedule (l-half LH=2048 -> 16 l-tiles of 128):
  phase 0: q = WqT^T @ xf (+bq) -> bf16 [C, L]
           k = WkT^T @ xh (+bk) -> bf16 [C, LH]
           vT_i = xh_i^T @ WvT (+bv) -> f32 [128, C] per l-tile
  main loop (per l-tile i), all engines pipelined by Tile:
    PE:  S_i = k_i^T @ q in four 1024-col chunks (bf16 in, f32 PSUM,
         2-chunk double buffer in PSUM banks 0-3), plus the previous
         tile's 8 attn matmuls into 4 persistent [128,1024] PSUM
         accumulators (banks 4-7)
    ACT: T_i chunk = exp(scale * S chunk) -> SBUF bf16
    DVE: Z_i = rowsum(T_i) (4 chunk reduces + combine), r = 1/Z,
         vts_i = vT_i * r  (bf16)
  tail: DVE-copy the 4 attn accumulators to SBUF, DMA out (f32).

All matmuls are bf16 (fp32 matmul runs LOW_HIGH double passes on TRN2 =
2x slower); x and the weights are pre-cast to bf16 on the host.
Softmax max-subtraction is skipped: scores*scale ~ N(0,1) here, so
exp() cannot overflow and softmax is shift-invariant anyway.
"""

import sys

if "/opt/trn_rl_repo" not in sys.path:
    sys.path.insert(0, "/opt/trn_rl_repo")

import numpy as np

N, C, H, W = 4, 128, 64, 64
L = H * W            # 4096
LH = L // 2          # 2048 l-half per core
P = 128              # partitions / l-tile size
NT = LH // P         # 16 l-tiles per core
BANK = 512           # fp32 elems per PSUM bank
CH = 1024            # S-chunk / attn-accumulator width (2 PSUM banks)
NCH = L // CH        # 4 chunks
NCORES = 8
SCALE = float(1.0 / np.sqrt(C))

_CACHE = {}


def _build_nc():
    import concourse.bass as bass
    import concourse.tile as tile
    from concourse import bacc, mybir
    from contextlib import ExitStack

    f32 = mybir.dt.float32
    bf16 = mybir.dt.bfloat16

    nc = bacc.Bacc("TRN2", target_bir_lowering=False, debug=False)

    xf = nc.dram_tensor("xf", [P, L], bf16, kind="ExternalInput").ap()
    xh = nc.dram_tensor("xh", [P, LH], bf16, kind="ExternalInput").ap()
    wqT = nc.dram_tensor("wqT", [P, P], bf16, kind="ExternalInput").ap()
    wkT = nc.dram_tensor("wkT", [P, P], bf16, kind="ExternalInput").ap()
    wvT = nc.dram_tensor("wvT", [P, P], bf16, kind="ExternalInput").ap()
    bq = nc.dram_tensor("bq", [P, 1], f32, kind="ExternalInput").ap()
    bk = nc.dram_tensor("bk", [P, 1], f32, kind="ExternalInput").ap()
    bv = nc.dram_tensor("bv", [1, P], f32, kind="ExternalInput").ap()
    attn_out = nc.dram_tensor("attn_part", [P, L], f32, kind="ExternalOutput").ap()

    Exp = mybir.ActivationFunctionType.Exp

    with tile.TileContext(nc) as tc, ExitStack() as ctx:
        const = ctx.enter_context(tc.tile_pool(name="const", bufs=1))
        persist = ctx.enter_context(tc.tile_pool(name="persist", bufs=1))

        wq_sb = const.tile([P, P], bf16)
        wk_sb = const.tile([P, P], bf16)
        wv_sb = const.tile([P, P], bf16)
        bq_sb = const.tile([P, 1], f32)
        bk_sb = const.tile([P, 1], f32)
        bv_sb = const.tile([P, P], f32)  # bv broadcast across partitions
        warm = const.tile([P, 1], f32)
        nc.sync.dma_start(out=wq_sb, in_=wqT)
        nc.sync.dma_start(out=wk_sb, in_=wkT)
        nc.sync.dma_start(out=wv_sb, in_=wvT)
        nc.sync.dma_start(out=bq_sb, in_=bq)
        nc.sync.dma_start(out=bk_sb, in_=bk)
        bv_bcast = bass.AP(tensor=bv.tensor, offset=bv.offset,
                           ap=[[0, P], bv.ap[1]])
        nc.sync.dma_start(out=bv_sb, in_=bv_bcast)
        # warm the ACT exp table while DMAs run (first exp otherwise pays
        # the ~2.7us ACT_TABLE_LOAD on the critical path)
        nc.scalar.activation(warm, bq_sb, Exp, scale=0.0)

        q_sb = persist.tile([P, L], bf16)
        k_sb = persist.tile([P, LH], bf16)
        vt_sb = persist.tile([P, NT, P], f32)   # [l, tile, c]
        vts = persist.tile([P, NT, P], bf16)    # vT * (1/Z), bf16
        z3 = persist.tile([P, NT, 3], f32)      # partial row sums of T
        zs = persist.tile([P, NT], f32)
        rs = persist.tile([P, NT], f32)
        attn_sb = persist.tile([P, L], f32)     # attn partial accumulator

        # ---- phase 0: projections -------------------------------------
        with tc.tile_pool(name="xp", bufs=1) as xp, \
             tc.tile_pool(name="p0ps", bufs=2, space="PSUM") as p0:
            x_sb = xp.tile([P, L], bf16)
            xh_sb = xp.tile([P, LH], bf16)
            # xh first (k/vT unlock early); split x so q can start early
            nc.sync.dma_start(out=xh_sb, in_=xh)
            nc.sync.dma_start(out=x_sb[:, :LH], in_=xf[:, :LH])
            nc.sync.dma_start(out=x_sb[:, LH:], in_=xf[:, LH:])

            Ident = mybir.ActivationFunctionType.Identity
            # q = WqT^T @ x + bq, two 2048-wide passes -> bf16
            # (PSUM->SBUF copies + bias ride on ScalarE, idle in phase 0)
            for h in range(2):
                t = p0.tile([P, LH], f32, tag="p0")
                for j in range(LH // BANK):
                    c0 = h * LH + j * BANK
                    nc.tensor.matmul(t[:, j * BANK:(j + 1) * BANK],
                                     wq_sb, x_sb[:, c0:c0 + BANK])
                nc.scalar.activation(q_sb[:, h * LH:(h + 1) * LH], t, Ident,
                                     bias=bq_sb)

            # k = WkT^T @ xh + bk -> bf16
            t = p0.tile([P, LH], f32, tag="p0")
            for j in range(LH // BANK):
                nc.tensor.matmul(t[:, j * BANK:(j + 1) * BANK],
                                 wk_sb, xh_sb[:, j * BANK:(j + 1) * BANK])
            nc.scalar.activation(k_sb, t, Ident, bias=bk_sb)

            # vT_i = xh_i^T @ WvT (+ bv broadcast along free dim) -> f32
            t = p0.tile([P, LH], f32, tag="p0")
            for i in range(NT):
                nc.tensor.matmul(t[:, i * P:(i + 1) * P],
                                 xh_sb[:, i * P:(i + 1) * P], wv_sb)
            for i in range(NT):
                nc.vector.tensor_add(vt_sb[:, i, :], t[:, i * P:(i + 1) * P], bv_sb)

        # ---- T storage (reuses the SBUF space freed by xp) -------------
        tpool = ctx.enter_context(tc.tile_pool(name="tpool", bufs=1))
        t_all = tpool.tile([P, NT, L], bf16)

        # ---- main loop: scores/softmax + interleaved attn matmuls ------
        # attn is accumulated in PSUM over groups of l-tiles, one
        # 1024-wide m-range sub-pass at a time (2 banks), then flushed
        # into attn_sb by DVE.  Group g's sub-passes are spread across
        # later tiles so PE stays fed while ACT runs exp; the last two
        # groups are half-size so the post-loop serial tail is short.
        # Z row-sums: chunks 0,1 ride the exp's ACT accumulator; chunks
        # 2,3 are one DVE reduce; vts scaling goes to GpSimd.
        GROUPS = [(0, 4), (4, 4), (8, 4), (12, 2), (14, 2)]  # (start, len)
        with tc.tile_pool(name="sps", bufs=2, space="PSUM") as sp, \
             tc.tile_pool(name="aps", bufs=2, space="PSUM") as ap, \
             tc.tile_pool(name="outp", bufs=2) as outp:

            def attn_sub_pass(g, sub):
                g0, glen = GROUPS[g]
                t = ap.tile([P, CH], f32, tag="acc", name="acc")
                for idx in range(glen):
                    i = g0 + idx
                    for hh in range(2):
                        m0 = sub * CH + hh * BANK
                        nc.tensor.matmul(t[:, hh * BANK:(hh + 1) * BANK],
                                         vts[:, i, :],
                                         t_all[:, i, m0:m0 + BANK],
                                         start=(idx == 0), stop=(idx == glen - 1))
                msl = slice(sub * CH, (sub + 1) * CH)
                if g == 0:
                    nc.vector.tensor_copy(attn_sb[:, msl], t)
                elif g < len(GROUPS) - 1:
                    nc.vector.tensor_add(attn_sb[:, msl], attn_sb[:, msl], t)
                else:
                    ao = outp.tile([P, CH], f32, tag="ao", name="ao")
                    nc.vector.tensor_add(ao, attn_sb[:, msl], t)
                    nc.sync.dma_start(out=attn_out[:, msl], in_=ao)

            # which (group, sub) attn passes to emit at tile i: group g's
            # 4 sub-passes spread over the tiles after the group completes
            attn_sched = {i: [] for i in range(NT)}
            attn_sched[4] = [(0, 0)]; attn_sched[5] = [(0, 1)]
            attn_sched[6] = [(0, 2)]; attn_sched[7] = [(0, 3)]
            attn_sched[8] = [(1, 0)]; attn_sched[9] = [(1, 1)]
            attn_sched[10] = [(1, 2)]; attn_sched[11] = [(1, 3)]
            attn_sched[12] = [(2, 0)]; attn_sched[13] = [(2, 1)]
            attn_sched[14] = [(2, 2), (3, 0)]
            attn_sched[15] = [(2, 3), (3, 1)]
            tail = [(3, 2), (3, 3), (4, 0), (4, 1), (4, 2), (4, 3)]

            def s_chunk(i, c, z_acc):
                s = sp.tile([P, CH], f32, tag="s")
                for j in range(CH // BANK):
                    m0 = c * CH + j * BANK
                    nc.tensor.matmul(s[:, j * BANK:(j + 1) * BANK],
                                     k_sb[:, i * P:(i + 1) * P],
                                     q_sb[:, m0:m0 + BANK])
                nc.scalar.activation(
                    t_all[:, i, c * CH:(c + 1) * CH], s, Exp, scale=SCALE,
                    accum_out=(z3[:, i, c:c + 1] if z_acc else None))

            for i in range(NT):
                passes = list(attn_sched[i])
                # chunk 0,1 first (exp+acc), attn MMs fill the PE stalls
                s_chunk(i, 0, True)
                s_chunk(i, 1, True)
                if passes:
                    attn_sub_pass(*passes[0])
                s_chunk(i, 2, False)
                if len(passes) > 1:
                    attn_sub_pass(*passes[1])
                s_chunk(i, 3, False)
                # Z tail on DVE: one reduce over chunks 2+3, combine, recip
                nc.vector.reduce_sum(out=z3[:, i, 2:3],
                                     in_=t_all[:, i, 2 * CH:4 * CH],
                                     axis=mybir.AxisListType.X)
                nc.vector.reduce_sum(out=zs[:, i:i + 1], in_=z3[:, i, :],
                                     axis=mybir.AxisListType.X)
                nc.vector.reciprocal(rs[:, i:i + 1], zs[:, i:i + 1])
                nc.gpsimd.tensor_scalar_mul(vts[:, i, :], vt_sb[:, i, :],
                                            rs[:, i:i + 1])
            for g, sub in tail:
                attn_sub_pass(g, sub)

    nc.compile()
    return nc


def _get_nc():
    if "nc" not in _CACHE:
        _CACHE["nc"] = _build_nc()
    return _CACHE["nc"]


def _make_in_maps(inputs):
    import ml_dtypes
    bf = ml_dtypes.bfloat16
    x = np.ascontiguousarray(np.asarray(inputs["x"], dtype=np.float32))
    wqT = np.ascontiguousarray(np.asarray(inputs["Wq"], dtype=np.float32).T.astype(bf))
    wkT = np.ascontiguousarray(np.asarray(inputs["Wk"], dtype=np.float32).T.astype(bf))
    wvT = np.ascontiguousarray(np.asarray(inputs["Wv"], dtype=np.float32).T.astype(bf))
    bq = np.ascontiguousarray(np.asarray(inputs["bq"], dtype=np.float32).reshape(P, 1))
    bk = np.ascontiguousarray(np.asarray(inputs["bk"], dtype=np.float32).reshape(P, 1))
    bv = np.ascontiguousarray(np.asarray(inputs["bv"], dtype=np.float32).reshape(1, P))
    in_maps = []
    for core in range(NCORES):
        n, half = core // 2, core % 2
        xf32 = x[n].reshape(C, L)
        xfb = np.ascontiguousarray(xf32.astype(bf))
        xhb = np.ascontiguousarray(xfb[:, half * LH:(half + 1) * LH])
        in_maps.append({
            "xf": xfb, "xh": xhb,
            "wqT": wqT, "wkT": wkT, "wvT": wvT,
            "bq": bq, "bk": bk, "bv": bv,
        })
    return in_maps, x


def run_on_hw(inputs, trace=False, **kwargs):
    """Returns (list of per-core attn_part arrays, BassKernelResults)."""
    from concourse import bass_utils
    nc = _get_nc()
    in_maps, _ = _make_in_maps(inputs)
    res = bass_utils.run_bass_kernel_spmd(
        nc, in_maps, list(range(NCORES)), trace=trace, **kwargs)
    parts = [res.results[i]["attn_part"] for i in range(NCORES)]
    return parts, res


def kernel(**inputs) -> np.ndarray:
    in_maps, x = _make_in_maps(inputs)
    parts, _ = run_on_hw(inputs)
    out = np.empty((N, C, H, W), dtype=np.float32)
    for n in range(N):
        attn = parts[2 * n] + parts[2 * n + 1]
        out[n] = x[n] + attn.reshape(C, H, W)
    return out


# revision 11
# speedup vs baseline: 1.1391x; 1.1391x over previous
"""Trainium2 Bass kernel for nn_ContextAttention_21457656611319.

Reference math (per batch n):
    xf = x[n] reshaped [C, L], L = H*W = 4096
    q = Wq@xf + bq ; k = Wk@xf + bk ; v = Wv@xf + bv          [C, L]
    S[l,m] = sum_c k[c,l] q[c,m] * (1/sqrt(C))                 [L, L]
    T = softmax(S, axis=m)  (softmax over the m axis)
    attn[c,m] = sum_l v[c,l] T[l,m]
    out = x + attn

Sharding: 8 cores = 4 batches x 2-way shard of the l (key/value) axis.
Softmax rows (fixed l, all m) stay intact on one core, so each core
computes a partial attn (partial sum over its l-half) independently;
the host adds the two halves per batch plus x.  No collectives needed.

Per-core schedule (l-half LH=2048 -> 16 l-tiles of 128):
  phase 0: q = WqT^T @ xf (+bq) -> bf16 [C, L]
           k = WkT^T @ xh (+bk) -> bf16 [C, LH]
           vT_i = xh_i^T @ WvT (+bv) -> f32 [128, C] per l-tile
  main loop (per l-tile i), all engines pipelined by Tile:
    PE:  S_i = k_i^T @ q in four 1024-col chunks (bf16 in, f32 PSUM,
         2-chunk double buffer in PSUM banks 0-3), plus the previous
         tile's 8 attn matmuls into 4 persistent [128,1024] PSUM
         accumulators (banks 4-7)
    ACT: T_i chunk = exp(scale * S chunk) -> SBUF bf16
    DVE: Z_i = rowsum(T_i) (4 chunk reduces + combine), r = 1/Z,
         vts_i = vT_i * r  (bf16)
  tail: DVE-copy the 4 attn accumulators to SBUF, DMA out (f32).

All matmuls are bf16 (fp32 matmul runs LOW_HIGH double passes on TRN2 =
2x slower); x and the weights are pre-cast to bf16 on the host.
Softmax max-subtraction is skipped: scores*scale ~ N(0,1) here, so
exp() cannot overflow and softmax is shift-invariant anyway.
"""

import sys

if "/opt/trn_rl_repo" not in sys.path:
    sys.path.insert(0, "/opt/trn_rl_repo")

import numpy as np

N, C, H, W = 4, 128, 64, 64
L = H * W            # 4096
LH = L // 2          # 2048 l-half per core
P = 128              # partitions / l-tile size
NT = LH // P         # 16 l-tiles per core
BANK = 512           # fp32 elems per PSUM bank
CH = 1024            # S-chunk / attn-accumulator width (2 PSUM banks)
NCH = L // CH        # 4 chunks
NCORES = 8
SCALE = float(1.0 / np.sqrt(C))

_CACHE = {}


def _build_nc():
    import concourse.bass as bass
    import concourse.tile as tile
    from concourse import bacc, mybir
    from contextlib import ExitStack

    f32 = mybir.dt.float32
    bf16 = mybir.dt.bfloat16

    nc = bacc.Bacc("TRN2", target_bir_lowering=False, debug=False)

    xf = nc.dram_tensor("xf", [P, L], bf16, kind="ExternalInput").ap()
    xh = nc.dram_tensor("xh", [P, LH], bf16, kind="ExternalInput").ap()
    wqT = nc.dram_tensor("wqT", [P, P], bf16, kind="ExternalInput").ap()
    wkT = nc.dram_tensor("wkT", [P, P], bf16, kind="ExternalInput").ap()
    wvT = nc.dram_tensor("wvT", [P, P], bf16, kind="ExternalInput").ap()
    bq = nc.dram_tensor("bq", [P, 1], f32, kind="ExternalInput").ap()
    bk = nc.dram_tensor("bk", [P, 1], f32, kind="ExternalInput").ap()
    bv = nc.dram_tensor("bv", [1, P], f32, kind="ExternalInput").ap()
    attn_out = nc.dram_tensor("attn_part", [P, L], f32, kind="ExternalOutput").ap()

    Exp = mybir.ActivationFunctionType.Exp

    with tile.TileContext(nc) as tc, ExitStack() as ctx:
        const = ctx.enter_context(tc.tile_pool(name="const", bufs=1))
        persist = ctx.enter_context(tc.tile_pool(name="persist", bufs=1))

        wq_sb = const.tile([P, P], bf16)
        wk_sb = const.tile([P, P], bf16)
        wv_sb = const.tile([P, P], bf16)
        bq_sb = const.tile([P, 1], f32)
        bk_sb = const.tile([P, 1], f32)
        bv_sb = const.tile([P, P], f32)  # bv broadcast across partitions
        warm = const.tile([P, 1], f32)
        nc.sync.dma_start(out=wq_sb, in_=wqT)
        nc.sync.dma_start(out=wk_sb, in_=wkT)
        nc.sync.dma_start(out=wv_sb, in_=wvT)
        nc.sync.dma_start(out=bq_sb, in_=bq)
        nc.sync.dma_start(out=bk_sb, in_=bk)
        bv_bcast = bass.AP(tensor=bv.tensor, offset=bv.offset,
                           ap=[[0, P], bv.ap[1]])
        nc.sync.dma_start(out=bv_sb, in_=bv_bcast)
        # warm the ACT exp table while DMAs run (first exp otherwise pays
        # the ~2.7us ACT_TABLE_LOAD on the critical path)
        nc.scalar.activation(warm, bq_sb, Exp, scale=0.0)

        q_sb = persist.tile([P, L], bf16)
        k_sb = persist.tile([P, LH], bf16)
        vt_sb = persist.tile([P, NT, P], f32)   # [l, tile, c]
        vts = persist.tile([P, NT, P], bf16)    # vT * (1/Z), bf16
        z3 = persist.tile([P, NT, 3], f32)      # partial row sums of T
        zs = persist.tile([P, NT], f32)
        rs = persist.tile([P, NT], f32)
        attn_sb = persist.tile([P, L], f32)     # attn partial accumulator

        # ---- phase 0: projections -------------------------------------
        with tc.tile_pool(name="xp", bufs=1) as xp, \
             tc.tile_pool(name="p0ps", bufs=2, space="PSUM") as p0:
            x_sb = xp.tile([P, L], bf16)
            xh_sb = xp.tile([P, LH], bf16)
            # xh first (k/vT unlock early); split x so q can start early
            nc.sync.dma_start(out=xh_sb, in_=xh)
            nc.sync.dma_start(out=x_sb[:, :LH], in_=xf[:, :LH])
            nc.sync.dma_start(out=x_sb[:, LH:], in_=xf[:, LH:])

            Ident = mybir.ActivationFunctionType.Identity
            # q = WqT^T @ x + bq, two 2048-wide passes -> bf16
            # (PSUM->SBUF copies + bias ride on ScalarE, idle in phase 0)
            for h in range(2):
                t = p0.tile([P, LH], f32, tag="p0")
                for j in range(LH // BANK):
                    c0 = h * LH + j * BANK
                    nc.tensor.matmul(t[:, j * BANK:(j + 1) * BANK],
                                     wq_sb, x_sb[:, c0:c0 + BANK])
                nc.scalar.activation(q_sb[:, h * LH:(h + 1) * LH], t, Ident,
                                     bias=bq_sb)

            # k = WkT^T @ xh + bk -> bf16
            t = p0.tile([P, LH], f32, tag="p0")
            for j in range(LH // BANK):
                nc.tensor.matmul(t[:, j * BANK:(j + 1) * BANK],
                                 wk_sb, xh_sb[:, j * BANK:(j + 1) * BANK])
            nc.scalar.activation(k_sb, t, Ident, bias=bk_sb)

            # vT_i = xh_i^T @ WvT (+ bv broadcast along free dim) -> f32
            t = p0.tile([P, LH], f32, tag="p0")
            for i in range(NT):
                nc.tensor.matmul(t[:, i * P:(i + 1) * P],
                                 xh_sb[:, i * P:(i + 1) * P], wv_sb)
            for i in range(NT):
                nc.vector.tensor_add(vt_sb[:, i, :], t[:, i * P:(i + 1) * P], bv_sb)

        # ---- T storage (reuses the SBUF space freed by xp) -------------
        tpool = ctx.enter_context(tc.tile_pool(name="tpool", bufs=1))
        t_all = tpool.tile([P, NT, L], bf16)

        # ---- main loop: scores/softmax + interleaved attn matmuls ------
        # attn is accumulated in PSUM over groups of l-tiles, one
        # 1024-wide m-range sub-pass at a time (2 banks), then flushed
        # into attn_sb by DVE.  Group g's sub-passes are spread across
        # later tiles so PE stays fed while ACT runs exp; the last two
        # groups are half-size so the post-loop serial tail is short.
        # Z row-sums: chunks 0,1 ride the exp's ACT accumulator; chunks
        # 2,3 are one DVE reduce; vts scaling goes to GpSimd.
        GROUPS = [(0, 4), (4, 4), (8, 4), (12, 2), (14, 2)]  # (start, len)
        with tc.tile_pool(name="sps", bufs=2, space="PSUM") as sp, \
             tc.tile_pool(name="aps", bufs=2, space="PSUM") as ap, \
             tc.tile_pool(name="outp", bufs=2) as outp:

            def attn_sub_pass(g, sub):
                g0, glen = GROUPS[g]
                t = ap.tile([P, CH], f32, tag="acc", name="acc")
                for idx in range(glen):
                    i = g0 + idx
                    for hh in range(2):
                        m0 = sub * CH + hh * BANK
                        nc.tensor.matmul(t[:, hh * BANK:(hh + 1) * BANK],
                                         vts[:, i, :],
                                         t_all[:, i, m0:m0 + BANK],
                                         start=(idx == 0), stop=(idx == glen - 1))
                msl = slice(sub * CH, (sub + 1) * CH)
                if g == 0:
                    nc.vector.tensor_copy(attn_sb[:, msl], t)
                elif g < len(GROUPS) - 1:
                    nc.vector.tensor_add(attn_sb[:, msl], attn_sb[:, msl], t)
                else:
                    ao = outp.tile([P, CH], f32, tag="ao", name="ao")
                    nc.vector.tensor_add(ao, attn_sb[:, msl], t)
                    nc.sync.dma_start(out=attn_out[:, msl], in_=ao)

            # which (group, sub) attn passes to emit at tile i: group g's
            # 4 sub-passes spread over the tiles after the group completes
            attn_sched = {i: [] for i in range(NT)}
            attn_sched[4] = [(0, 0)]; attn_sched[5] = [(0, 1)]
            attn_sched[6] = [(0, 2)]; attn_sched[7] = [(0, 3)]
            attn_sched[8] = [(1, 0)]; attn_sched[9] = [(1, 1)]
            attn_sched[10] = [(1, 2)]; attn_sched[11] = [(1, 3)]
            attn_sched[12] = [(2, 0)]; attn_sched[13] = [(2, 1)]
            attn_sched[14] = [(2, 2), (3, 0)]
            attn_sched[15] = [(2, 3), (3, 1)]
            tail = [(3, 2), (3, 3), (4, 0), (4, 1), (4, 2), (4, 3)]

            def s_chunk(i, c, z_acc):
                s = sp.tile([P, CH], f32, tag="s")
                for j in range(CH // BANK):
                    m0 = c * CH + j * BANK
                    nc.tensor.matmul(s[:, j * BANK:(j + 1) * BANK],
                                     k_sb[:, i * P:(i + 1) * P],
                                     q_sb[:, m0:m0 + BANK])
                nc.scalar.activation(
                    t_all[:, i, c * CH:(c + 1) * CH], s, Exp, scale=SCALE,
                    accum_out=(z3[:, i, c:c + 1] if z_acc else None))

            for i in range(NT):
                passes = list(attn_sched[i])
                # chunk 0,1 first (exp+acc), attn MMs fill the PE stalls
                s_chunk(i, 0, True)
                s_chunk(i, 1, True)
                if passes:
                    attn_sub_pass(*passes[0])
                s_chunk(i, 2, False)
                if len(passes) > 1:
                    attn_sub_pass(*passes[1])
                s_chunk(i, 3, False)
                # Z tail on DVE: one reduce over chunks 2+3, combine, recip
                nc.vector.reduce_sum(out=z3[:, i, 2:3],
                                     in_=t_all[:, i, 2 * CH:4 * CH],
                                     axis=mybir.AxisListType.X)
                nc.vector.reduce_sum(out=zs[:, i:i + 1], in_=z3[:, i, :],
                                     axis=mybir.AxisListType.X)
                nc.vector.reciprocal(rs[:, i:i + 1], zs[:, i:i + 1])
                nc.vector.tensor_scalar_mul(vts[:, i, :], vt_sb[:, i, :],
                                            rs[:, i:i + 1])
            for g, sub in tail:
                attn_sub_pass(g, sub)

    nc.compile()
    return nc


def _get_nc():
    if "nc" not in _CACHE:
        _CACHE["nc"] = _build_nc()
    return _CACHE["nc"]


def _make_in_maps(inputs):
    import ml_dtypes
    bf = ml_dtypes.bfloat16
    x = np.ascontiguousarray(np.asarray(inputs["x"], dtype=np.float32))
    wqT = np.ascontiguousarray(np.asarray(inputs["Wq"], dtype=np.float32).T.astype(bf))
    wkT = np.ascontiguousarray(np.asarray(inputs["Wk"], dtype=np.float32).T.astype(bf))
    wvT = np.ascontiguousarray(np.asarray(inputs["Wv"], dtype=np.float32).T.astype(bf))
    bq = np.ascontiguousarray(np.asarray(inputs["bq"], dtype=np.float32).reshape(P, 1))
    bk = np.ascontiguousarray(np.asarray(inputs["bk"], dtype=np.float32).reshape(P, 1))
    bv = np.ascontiguousarray(np.asarray(inputs["bv"], dtype=np.float32).reshape(1, P))
    in_maps = []
    for core in range(NCORES):
        n, half = core // 2, core % 2
        xf32 = x[n].reshape(C, L)
        xfb = np.ascontiguousarray(xf32.astype(bf))
        xhb = np.ascontiguousarray(xfb[:, half * LH:(half + 1) * LH])
        in_maps.append({
            "xf": xfb, "xh": xhb,
            "wqT": wqT, "wkT": wkT, "wvT": wvT,
            "bq": bq, "bk": bk, "bv": bv,
        })
    return in_maps, x


def run_on_hw(inputs, trace=False, **kwargs):
    """Returns (list of per-core attn_part arrays, BassKernelResults)."""
    from concourse import bass_utils
    nc = _get_nc()
    in_maps, _ = _make_in_maps(inputs)
    res = bass_utils.run_bass_kernel_spmd(
        nc, in_maps, list(range(NCORES)), trace=trace, **kwargs)
    parts = [res.results[i]["attn_part"] for i in range(NCORES)]
    return parts, res


def kernel(**inputs) -> np.ndarray:
    in_maps, x = _make_in_maps(inputs)
    parts, _ = run_on_hw(inputs)
    out = np.empty((N, C, H, W), dtype=np.float32)
    for n in range(N):
        attn = parts[2 * n] + parts[2 * n + 1]
        out[n] = x[n] + attn.reshape(C, H, W)
    return out


# revision 15
# speedup vs baseline: 1.2228x; 1.0735x over previous
"""Trainium2 Bass kernel for nn_ContextAttention_21457656611319.

Reference math (per batch n):
    xf = x[n] reshaped [C, L], L = H*W = 4096
    q = Wq@xf + bq ; k = Wk@xf + bk ; v = Wv@xf + bv          [C, L]
    S[l,m] = sum_c k[c,l] q[c,m] * (1/sqrt(C))                 [L, L]
    T = softmax(S, axis=m)  (softmax over the m axis)
    attn[c,m] = sum_l v[c,l] T[l,m]
    out = x + attn

Sharding: 8 cores = 4 batches x 2-way shard of the l (key/value) axis.
Softmax rows (fixed l, all m) stay intact on one core, so each core
computes a partial attn (partial sum over its l-half) independently;
the host adds the two halves per batch plus x.  No collectives needed.

Per-core schedule (l-half LH=2048 -> 16 l-tiles of 128):
  phase 0: q = WqT^T @ xf (+bq) -> bf16 [C, L]
           k = WkT^T @ xh (+bk) -> bf16 [C, LH]
           vT_i = xh_i^T @ WvT (+bv) -> f32 [128, C] per l-tile
  main loop (per l-tile i), all engines pipelined by Tile:
    PE:  S_i = k_i^T @ q in four 1024-col chunks (bf16 in, f32 PSUM,
         2-chunk double buffer in PSUM banks 0-3), plus the previous
         tile's 8 attn matmuls into 4 persistent [128,1024] PSUM
         accumulators (banks 4-7)
    ACT: T_i chunk = exp(scale * S chunk) -> SBUF bf16
    DVE: Z_i = rowsum(T_i) (4 chunk reduces + combine), r = 1/Z,
         vts_i = vT_i * r  (bf16)
  tail: DVE-copy the 4 attn accumulators to SBUF, DMA out (f32).

All matmuls are bf16 (fp32 matmul runs LOW_HIGH double passes on TRN2 =
2x slower); x and the weights are pre-cast to bf16 on the host.
Softmax max-subtraction is skipped: scores*scale ~ N(0,1) here, so
exp() cannot overflow and softmax is shift-invariant anyway.
"""

import sys

if "/opt/trn_rl_repo" not in sys.path:
    sys.path.insert(0, "/opt/trn_rl_repo")

import numpy as np

N, C, H, W = 4, 128, 64, 64
L = H * W            # 4096
LH = L // 2          # 2048 l-half per core
P = 128              # partitions / l-tile size
NT = LH // P         # 16 l-tiles per core
BANK = 512           # fp32 elems per PSUM bank
CH = 1024            # S-chunk / attn-accumulator width (2 PSUM banks)
NCH = L // CH        # 4 chunks
NCORES = 8
SCALE = float(1.0 / np.sqrt(C))

_CACHE = {}


def _build_nc():
    import concourse.bass as bass
    import concourse.tile as tile
    from concourse import bacc, mybir
    from contextlib import ExitStack

    f32 = mybir.dt.float32
    bf16 = mybir.dt.bfloat16

    nc = bacc.Bacc("TRN2", target_bir_lowering=False, debug=False)

    xf = nc.dram_tensor("xf", [P, L], bf16, kind="ExternalInput").ap()
    xh = nc.dram_tensor("xh", [P, LH], bf16, kind="ExternalInput").ap()
    wqT = nc.dram_tensor("wqT", [P, P], bf16, kind="ExternalInput").ap()
    wkT = nc.dram_tensor("wkT", [P, P], bf16, kind="ExternalInput").ap()
    wvT = nc.dram_tensor("wvT", [P, P], bf16, kind="ExternalInput").ap()
    bq = nc.dram_tensor("bq", [P, 1], f32, kind="ExternalInput").ap()
    bk = nc.dram_tensor("bk", [P, 1], f32, kind="ExternalInput").ap()
    bv = nc.dram_tensor("bv", [1, P], f32, kind="ExternalInput").ap()
    attn_out = nc.dram_tensor("attn_part", [P, L], f32, kind="ExternalOutput").ap()

    Exp = mybir.ActivationFunctionType.Exp

    with tile.TileContext(nc) as tc, ExitStack() as ctx:
        const = ctx.enter_context(tc.tile_pool(name="const", bufs=1))
        persist = ctx.enter_context(tc.tile_pool(name="persist", bufs=1))

        wq_sb = const.tile([P, P], bf16)
        wk_sb = const.tile([P, P], bf16)
        wv_sb = const.tile([P, P], bf16)
        bq_sb = const.tile([P, 1], f32)
        bk_sb = const.tile([P, 1], f32)
        bv_sb = const.tile([P, P], f32)  # bv broadcast across partitions
        warm = const.tile([P, 1], f32)
        nc.sync.dma_start(out=wq_sb, in_=wqT)
        nc.sync.dma_start(out=wk_sb, in_=wkT)
        nc.sync.dma_start(out=wv_sb, in_=wvT)
        nc.sync.dma_start(out=bq_sb, in_=bq)
        nc.sync.dma_start(out=bk_sb, in_=bk)
        bv_bcast = bass.AP(tensor=bv.tensor, offset=bv.offset,
                           ap=[[0, P], bv.ap[1]])
        nc.sync.dma_start(out=bv_sb, in_=bv_bcast)
        # warm the ACT exp table while DMAs run (first exp otherwise pays
        # the ~2.7us ACT_TABLE_LOAD on the critical path)
        nc.scalar.activation(warm, bq_sb, Exp, scale=0.0)

        q_sb = persist.tile([P, L], bf16)
        k_sb = persist.tile([P, LH], bf16)
        vt_sb = persist.tile([P, NT, P], f32)   # [l, tile, c]
        vts = persist.tile([P, NT, P], bf16)    # vT * (1/Z), bf16
        z3 = persist.tile([P, NT, 3], f32)      # partial row sums of T
        zs = persist.tile([P, NT], f32)
        rs = persist.tile([P, NT], f32)
        attn_sb = persist.tile([P, L], f32)     # attn partial accumulator

        # ---- main loop: scores/softmax + interleaved attn matmuls ------
        # attn is accumulated in PSUM over groups of l-tiles, one
        # 1024-wide m-range sub-pass at a time (2 banks), then flushed
        # into attn_sb by DVE.  Group g's sub-passes are spread across
        # later tiles so PE stays fed while ACT runs exp; the last two
        # groups are half-size so the post-loop serial tail is short.
        # Z row-sums: chunks 0,1 ride the exp's ACT accumulator; chunks
        # 2,3 are one DVE reduce; vts scaling goes to GpSimd.
        GROUPS = [(0, 4), (4, 4), (8, 4), (12, 2), (14, 2)]  # (start, len)
        with tc.tile_pool(name="sps", bufs=2, space="PSUM") as sp, \
             tc.tile_pool(name="aps", bufs=2, space="PSUM") as ap, \
             tc.tile_pool(name="outp", bufs=2) as outp:

            # ---- projections, streamed through the same PSUM pools ----
            # (no separate phase-0 pool: avoids the PSUM pool-release
            # barrier that kept the first score matmul ~30us out)
            Ident = mybir.ActivationFunctionType.Identity
            with tc.tile_pool(name="xp", bufs=1) as xp:
                x_sb = xp.tile([P, L], bf16)
                xh_sb = xp.tile([P, LH], bf16)
                nc.sync.dma_start(out=xh_sb[:, :LH // 2], in_=xh[:, :LH // 2])
                nc.sync.dma_start(out=xh_sb[:, LH // 2:], in_=xh[:, LH // 2:])
                for h in range(4):
                    msl = slice(h * CH, (h + 1) * CH)
                    nc.sync.dma_start(out=x_sb[:, msl], in_=xf[:, msl])

                def q_pass(h):
                    t = sp.tile([P, CH], f32, tag="s", name="qp")
                    for j in range(CH // BANK):
                        c0 = h * CH + j * BANK
                        nc.tensor.matmul(t[:, j * BANK:(j + 1) * BANK],
                                         wq_sb, x_sb[:, c0:c0 + BANK])
                    nc.scalar.activation(q_sb[:, h * CH:(h + 1) * CH], t,
                                         Ident, bias=bq_sb)

                def k_pass(h):
                    t = sp.tile([P, CH], f32, tag="s", name="kp")
                    for j in range(CH // BANK):
                        c0 = h * CH + j * BANK
                        nc.tensor.matmul(t[:, j * BANK:(j + 1) * BANK],
                                         wk_sb, xh_sb[:, c0:c0 + BANK])
                    nc.scalar.activation(k_sb[:, h * CH:(h + 1) * CH], t,
                                         Ident, bias=bk_sb)

                def vt_pass(h):
                    t = ap.tile([P, CH], f32, tag="acc", name="vtp")
                    for j in range(CH // P):
                        i = h * (CH // P) + j
                        nc.tensor.matmul(t[:, j * P:(j + 1) * P],
                                         xh_sb[:, i * P:(i + 1) * P], wv_sb)
                    for j in range(CH // P):
                        i = h * (CH // P) + j
                        nc.vector.tensor_add(vt_sb[:, i, :],
                                             t[:, j * P:(j + 1) * P], bv_sb)

                q_pass(0); k_pass(0); vt_pass(0)
                q_pass(1); k_pass(1); vt_pass(1)
                q_pass(2); q_pass(3)

            # T storage reuses the SBUF space freed by xp
            tpool = tc.alloc_tile_pool(name="tpool", bufs=1)
            t_all = tpool.tile([P, NT, L], bf16)

            def attn_sub_pass(g, sub):
                g0, glen = GROUPS[g]
                t = ap.tile([P, CH], f32, tag="acc", name="acc")
                for idx in range(glen):
                    i = g0 + idx
                    for hh in range(2):
                        m0 = sub * CH + hh * BANK
                        nc.tensor.matmul(t[:, hh * BANK:(hh + 1) * BANK],
                                         vts[:, i, :],
                                         t_all[:, i, m0:m0 + BANK],
                                         start=(idx == 0), stop=(idx == glen - 1))
                msl = slice(sub * CH, (sub + 1) * CH)
                if g == 0:
                    nc.vector.tensor_copy(attn_sb[:, msl], t)
                elif g < len(GROUPS) - 1:
                    nc.vector.tensor_add(attn_sb[:, msl], attn_sb[:, msl], t)
                else:
                    ao = outp.tile([P, CH], f32, tag="ao", name="ao")
                    nc.vector.tensor_add(ao, attn_sb[:, msl], t)
                    nc.sync.dma_start(out=attn_out[:, msl], in_=ao)

            # which (group, sub) attn passes to emit at tile i: group g's
            # 4 sub-passes spread over the tiles after the group completes
            attn_sched = {i: [] for i in range(NT)}
            attn_sched[4] = [(0, 0)]; attn_sched[5] = [(0, 1)]
            attn_sched[6] = [(0, 2)]; attn_sched[7] = [(0, 3)]
            attn_sched[8] = [(1, 0)]; attn_sched[9] = [(1, 1)]
            attn_sched[10] = [(1, 2)]; attn_sched[11] = [(1, 3)]
            attn_sched[12] = [(2, 0)]; attn_sched[13] = [(2, 1)]
            attn_sched[14] = [(2, 2), (3, 0)]
            attn_sched[15] = [(2, 3), (3, 1)]
            tail = [(3, 2), (3, 3), (4, 0), (4, 1), (4, 2), (4, 3)]

            def s_chunk(i, c, z_acc):
                s = sp.tile([P, CH], f32, tag="s")
                for j in range(CH // BANK):
                    m0 = c * CH + j * BANK
                    nc.tensor.matmul(s[:, j * BANK:(j + 1) * BANK],
                                     k_sb[:, i * P:(i + 1) * P],
                                     q_sb[:, m0:m0 + BANK])
                nc.scalar.activation(
                    t_all[:, i, c * CH:(c + 1) * CH], s, Exp, scale=SCALE,
                    accum_out=(z3[:, i, c:c + 1] if z_acc else None))

            for i in range(NT):
                passes = list(attn_sched[i])
                # chunk 0,1 first (exp+acc), attn MMs fill the PE stalls
                s_chunk(i, 0, True)
                s_chunk(i, 1, True)
                if passes:
                    attn_sub_pass(*passes[0])
                s_chunk(i, 2, False)
                if len(passes) > 1:
                    attn_sub_pass(*passes[1])
                s_chunk(i, 3, False)
                # Z tail on DVE: one reduce over chunks 2+3, combine, recip
                nc.vector.reduce_sum(out=z3[:, i, 2:3],
                                     in_=t_all[:, i, 2 * CH:4 * CH],
                                     axis=mybir.AxisListType.X)
                nc.vector.reduce_sum(out=zs[:, i:i + 1], in_=z3[:, i, :],
                                     axis=mybir.AxisListType.X)
                nc.vector.reciprocal(rs[:, i:i + 1], zs[:, i:i + 1])
                nc.vector.tensor_scalar_mul(vts[:, i, :], vt_sb[:, i, :],
                                            rs[:, i:i + 1])
            for g, sub in tail:
                attn_sub_pass(g, sub)
            tpool.release()

    nc.compile()
    return nc


def _get_nc():
    if "nc" not in _CACHE:
        _CACHE["nc"] = _build_nc()
    return _CACHE["nc"]


def _make_in_maps(inputs):
    import ml_dtypes
    bf = ml_dtypes.bfloat16
    x = np.ascontiguousarray(np.asarray(inputs["x"], dtype=np.float32))
    wqT = np.ascontiguousarray(np.asarray(inputs["Wq"], dtype=np.float32).T.astype(bf))
    wkT = np.ascontiguousarray(np.asarray(inputs["Wk"], dtype=np.float32).T.astype(bf))
    wvT = np.ascontiguousarray(np.asarray(inputs["Wv"], dtype=np.float32).T.astype(bf))
    bq = np.ascontiguousarray(np.asarray(inputs["bq"], dtype=np.float32).reshape(P, 1))
    bk = np.ascontiguousarray(np.asarray(inputs["bk"], dtype=np.float32).reshape(P, 1))
    bv = np.ascontiguousarray(np.asarray(inputs["bv"], dtype=np.float32).reshape(1, P))
    in_maps = []
    for core in range(NCORES):
        n, half = core // 2, core % 2
        xf32 = x[n].reshape(C, L)
        xfb = np.ascontiguousarray(xf32.astype(bf))
        xhb = np.ascontiguousarray(xfb[:, half * LH:(half + 1) * LH])
        in_maps.append({
            "xf": xfb, "xh": xhb,
            "wqT": wqT, "wkT": wkT, "wvT": wvT,
            "bq": bq, "bk": bk, "bv": bv,
        })
    return in_maps, x


def run_on_hw(inputs, trace=False, **kwargs):
    """Returns (list of per-core attn_part arrays, BassKernelResults)."""
    from concourse import bass_utils
    nc = _get_nc()
    in_maps, _ = _make_in_maps(inputs)
    res = bass_utils.run_bass_kernel_spmd(
        nc, in_maps, list(range(NCORES)), trace=trace, **kwargs)
    parts = [res.results[i]["attn_part"] for i in range(NCORES)]
    return parts, res


def kernel(**inputs) -> np.ndarray:
    in_maps, x = _make_in_maps(inputs)
    parts, _ = run_on_hw(inputs)
    out = np.empty((N, C, H, W), dtype=np.float32)
    for n in range(N):
        attn = parts[2 * n] + parts[2 * n + 1]
        out[n] = x[n] + attn.reshape(C, H, W)
    return out
